# revision 22
# baseline (speedup 1.0000x reference)
"""Block-causal attention (B=2, S=2048, D=1024, H=16, HD=64, BLOCK=16) on 8 TRN2 cores.

Sharding: core c -> batch c//4, head-group c%4 (4 heads). Each core computes the
full attention for its 4 heads plus a partial out-projection y^T (1024, 2048);
the host sums the 4 partials per batch (row-parallel unshard) and transposes.

Device dataflow (per core) is fully "transposed":
  - qkv proj emits q^T/k^T in (head-dim, seq) layout, V in (seq, head-dim).
  - RMS-norm on q^T/k^T: squares on ACT, partition-sum via ones-matmul on PE,
    rsqrt as exp(-0.5*ln(.)) so Ln/Exp/Square share one ACT table set.
  - scores^T = (K^T-tile).T @ Q^T per head; block-causal mask is added inside
    the PE accumulation as a rank-8 (-720 * disallowed) matmul; exp needs no
    row-max because |scores| <= 8 after RMS norm (host passes the bound).
  - softmax denominator comes free: V carries an appended ones column (M=65).
  - attn^T = [V|1].T @ P^T accumulated over k-tiles; normalize by the
    reciprocal of row 64.
"""

import numpy as np
import ml_dtypes

import concourse.bass as bass
import concourse.tile as tile
from concourse import bacc
from concourse import mybir
from concourse.bass_utils import run_bass_kernel_spmd

BF16 = ml_dtypes.bfloat16
F32 = mybir.dt.float32
BF = mybir.dt.bfloat16

B, S, D, H, HD = 2, 2048, 1024, 16, 64
HLOC = 4          # heads per core
NCORES = 8
EPS = 1e-6
SCALE = HD ** -0.5
MASK_C = 8192.0   # masked-pair score offset; exp underflows to 0.0
NST = 4           # 512-wide seq tiles
NKT = 16          # 128-wide key tiles
NDK = 8           # 128-wide model-dim tiles


def _declare_io(nc):
    def din(name, shape, d=BF):
        return nc.dram_tensor(name, shape, d, kind="ExternalInput").ap()

    io = dict(
        xt_d=din("xt", [D, S]),
        wq_d=din("wq", [128, NDK * 256]),
        wk_d=din("wk", [128, NDK * 256]),
        wv_d=din("wv", [128, NDK * 256]),
        wo_d=din("wo", [128, 2 * D]),
        csq_d=din("csq", [128, S]),
        snq_d=din("snq", [128, S]),
        csk_d=din("csk", [128, S]),
        snk_d=din("snk", [128, S]),
        mu_d=din("mu", [8, 128]),
        mv_d=din("mv", [8, 128]),
        ones2_d=din("ones2", [128, 2]),
        b0_d=din("b0", [128, 1], F32),
        yt_d=nc.dram_tensor(
            "yt", [32, 128, 512], F32, kind="ExternalOutput"
        ).ap(),
    )
    return io


def _emit(tc, io, u=""):
    """Emit the per-core program. Pure SPMD: identical on all 8 cores.
    `u` suffixes pool names so the body can be emitted multiple times
    (benchmarking builds)."""
    from contextlib import ExitStack

    nc = tc.nc
    A = mybir.ActivationFunctionType
    OP = mybir.AluOpType
    xt_d = io["xt_d"]
    wq_d = io["wq_d"]
    wk_d = io["wk_d"]
    wv_d = io["wv_d"]
    wo_d = io["wo_d"]
    csq_d = io["csq_d"]
    snq_d = io["snq_d"]
    csk_d = io["csk_d"]
    snk_d = io["snk_d"]
    mu_d = io["mu_d"]
    mv_d = io["mv_d"]
    ones2_d = io["ones2_d"]
    b0_d = io["b0_d"]
    yt_d = io["yt_d"]

    ctx = ExitStack()
    proj_ctx = ExitStack()
    with ctx:
        consts = ctx.enter_context(tc.tile_pool(name="consts" + u, bufs=1))
        persist = ctx.enter_context(tc.tile_pool(name="persist" + u, bufs=1))
        dscratch = ctx.enter_context(tc.tile_pool(name="dscratch" + u, bufs=1, space="DRAM"))
        xtp = proj_ctx.enter_context(tc.tile_pool(name="xtp" + u, bufs=1))
        work2 = proj_ctx.enter_context(tc.tile_pool(name="work2" + u, bufs=2))
        sqp = proj_ctx.enter_context(tc.tile_pool(name="sqp" + u, bufs=3))
        pp = proj_ctx.enter_context(tc.tile_pool(name="pp" + u, bufs=2, space="PSUM"))
        vp = proj_ctx.enter_context(tc.tile_pool(name="vp" + u, bufs=2, space="PSUM"))
        msp = proj_ctx.enter_context(tc.tile_pool(name="msp" + u, bufs=1, space="PSUM"))

        # ---- input loads: x^T first (everything contracts over it), the
        # weights/tables on the gpsimd queue so both DMA channels fill ----
        xt_sb = xtp.tile([128, NDK, S], BF)
        for kt in range(NDK):
            nc.sync.dma_start(
                out=xt_sb[:, kt, :], in_=xt_d[128 * kt : 128 * (kt + 1), :]
            )
        wq_sb = consts.tile([128, NDK, 256], BF)
        wk_sb = consts.tile([128, NDK, 256], BF)
        wv_sb = consts.tile([128, NDK, 256], BF)
        wo_sb = consts.tile([128, 2, D], BF)
        nc.gpsimd.dma_start(out=wv_sb, in_=wv_d.rearrange("p (t m) -> p t m", t=NDK))
        nc.gpsimd.dma_start(out=wq_sb, in_=wq_d.rearrange("p (t m) -> p t m", t=NDK))
        nc.gpsimd.dma_start(out=wk_sb, in_=wk_d.rearrange("p (t m) -> p t m", t=NDK))
        csq_sb = consts.tile([128, S], BF)
        snq_sb = consts.tile([128, S], BF)
        csk_sb = consts.tile([128, S], BF)
        snk_sb = consts.tile([128, S], BF)
        nc.gpsimd.dma_start(out=csq_sb, in_=csq_d)
        nc.gpsimd.dma_start(out=snq_sb, in_=snq_d)
        nc.gpsimd.dma_start(out=csk_sb, in_=csk_d)
        nc.gpsimd.dma_start(out=snk_sb, in_=snk_d)
        nc.gpsimd.dma_start(out=wo_sb, in_=wo_d.rearrange("p (t m) -> p t m", t=2))
        mu_sb = consts.tile([8, 128], BF)
        mv_sb = consts.tile([8, 128], BF)
        nc.sync.dma_start(out=mu_sb, in_=mu_d)
        nc.sync.dma_start(out=mv_sb, in_=mv_d)
        ones2_sb = consts.tile([128, 2], BF)
        nc.sync.dma_start(out=ones2_sb, in_=ones2_d)
        b0_sb = consts.tile([128, 1], F32)
        nc.sync.dma_start(out=b0_sb, in_=b0_d)
        eps_sb = consts.tile([128, 1], F32)
        nc.vector.memset(eps_sb, EPS)

        # ---- persistent activations ----
        qT = persist.tile([128, 2, S], BF)      # (2 heads)*64 rows per m-tile
        kT = persist.tile([128, 2, S], BF)
        vv = persist.tile([128, NKT, HLOC, HD + 1], BF)   # [V | ones]
        at = persist.tile([128, 2, S], BF)      # normalized attn^T
        # pair p's two rows live at partition 32*p (engines need 32-aligned
        # start partitions)
        ln8 = persist.tile([98, NST, 512], F32)
        rr8 = persist.tile([98, NST, 512], BF)
        rkb = persist.tile([128, 64], BF)    # k-side rrms, (k mod 128, h*16+i)
        rkz = persist.tile([128, 4, 16], F32)  # SCALE * rrms_k per (head, ktile)
        rr_dram = dscratch.tile([8, 16, 128], BF)

        nc.vector.memset(vv[:, :, :, HD : HD + 1], 1.0)

        # ---- phase 1: V projection (s, d) ----
        for st in range(NKT):  # 128-row seq tiles
            ps = vp.tile([128, 256], F32, tag="vp")
            for kt in range(NDK):
                nc.tensor.matmul(
                    ps,
                    lhsT=xt_sb[:, kt, 128 * st : 128 * (st + 1)],
                    rhs=wv_sb[:, kt, :],
                    start=(kt == 0),
                    stop=(kt == NDK - 1),
                )
            nc.vector.tensor_copy(
                vv[:, st, :, 0:HD], ps.rearrange("p (h d) -> p h d", h=HLOC)
            )

        # ---- phase 2: Q/K projection + RMS norm + RoPE, (d, s) layout ----
        # RoPE commutes with the per-position rrms scale, and qn_w/kn_w are
        # folded into the cos/sin tables on the host, so the rotation DMA and
        # the rope muls run directly on the raw projection output; rrms is
        # applied last (Q side only -- the K side folds it into the exp
        # scale during attention).
        for qk_i, (wsb, cstab, sntab, dest) in enumerate(
            [(wq_sb, csq_sb, snq_sb, qT), (wk_sb, csk_sb, snk_sb, kT)]
        ):
            for mt in range(2):
                pair = qk_i * 2 + mt
                qraw = work2.tile([128, S], BF, tag="qraw")
                ms = msp.tile([2, NST, 512], F32, tag="msp")
                for st in range(NST):
                    ps = pp.tile([128, 512], F32, tag="pp")
                    for kt in range(NDK):
                        nc.tensor.matmul(
                            ps,
                            lhsT=wsb[:, kt, 128 * mt : 128 * (mt + 1)],
                            rhs=xt_sb[:, kt, 512 * st : 512 * (st + 1)],
                            start=(kt == 0),
                            stop=(kt == NDK - 1),
                        )
                    sl = slice(512 * st, 512 * (st + 1))
                    nc.vector.tensor_copy(qraw[:, sl], ps)
                    sq = sqp.tile([128, 512], BF, tag="sq")
                    nc.vector.tensor_mul(sq, qraw[:, sl], qraw[:, sl])
                    nc.tensor.matmul(
                        ms[:, st, :], lhsT=ones2_sb, rhs=sq, start=True, stop=True
                    )
                # rrms = exp(-0.5 * ln(ms/HD + eps)) for the 2 heads
                pb = 32 * pair
                nc.scalar.activation(
                    ln8[pb : pb + 2],
                    ms,
                    A.Ln,
                    bias=eps_sb[0:2],
                    scale=1.0 / HD,
                )
                nc.scalar.activation(
                    rr8[pb : pb + 2],
                    ln8[pb : pb + 2],
                    A.Exp,
                    scale=-0.5,
                )
                # rope on raw values (tables carry qn/kn and the sign fold)
                rot = work2.tile([128, S], BF, tag="rot")
                for lo, hi in ((0, 32), (32, 64), (64, 96), (96, 128)):
                    src_lo = lo + 32 if (lo // 32) % 2 == 0 else lo - 32
                    nc.sync.dma_start(
                        out=rot[lo:hi], in_=qraw[src_lo : src_lo + 32]
                    )
                t1 = work2.tile([128, S], BF, tag="t1")
                t2 = work2.tile([128, S], BF, tag="t2")
                nc.vector.tensor_mul(t1, qraw, cstab)
                nc.vector.tensor_mul(t2, rot, sntab)
                nc.sync.dma_start(
                    out=rr_dram[2 * pair : 2 * pair + 2].rearrange(
                        "r a b -> r (a b)"
                    ),
                    in_=rr8[pb : pb + 2].rearrange("p a b -> p (a b)"),
                )
                if qk_i == 1:
                    # K side: done after the add; rrms_k applied at exp time
                    nc.vector.tensor_add(dest[:, mt, :], t1, t2)
                    # k-side rrms rows -> partition-major via DMA transpose,
                    # folding in the 1/sqrt(HD) softmax scale
                    nc.sync.dma_start_transpose(
                        rkb[:, 32 * mt : 32 * (mt + 1)],
                        rr_dram[2 * pair : 2 * pair + 2].rearrange(
                            "r a b -> (r a) b"
                        ),
                    )
                    nc.vector.tensor_scalar_mul(
                        rkz[:, 2 * mt : 2 * mt + 2, :].rearrange(
                            "p h i -> p (h i)"
                        ),
                        rkb[:, 32 * mt : 32 * (mt + 1)],
                        SCALE,
                    )
                else:
                    tsum = work2.tile([128, S], BF, tag="tsum")
                    nc.vector.tensor_add(tsum, t1, t2)
                    # broadcast rrms across each head's 64 partitions
                    # (via DRAM: zero-step partition reads need a DRAM source)
                    rrb = work2.tile([128, NST, 512], BF, tag="rrb")
                    nc.gpsimd.dma_start(
                        out=rrb[0:64],
                        in_=rr_dram[2 * pair : 2 * pair + 1]
                        .rearrange("r a b -> r (a b)")
                        .rearrange("r (a b) -> r a b", a=NST)
                        .partition_broadcast(64),
                    )
                    nc.gpsimd.dma_start(
                        out=rrb[64:128],
                        in_=rr_dram[2 * pair + 1 : 2 * pair + 2]
                        .rearrange("r a b -> r (a b)")
                        .rearrange("r (a b) -> r a b", a=NST)
                        .partition_broadcast(64),
                    )
                    for st in range(NST):
                        sl = slice(512 * st, 512 * (st + 1))
                        nc.vector.tensor_mul(
                            dest[:, mt, sl], tsum[:, sl], rrb[:, st, :]
                        )

        # proj scratch (incl. x^T) is dead now; free SBUF/PSUM for attention
        proj_ctx.close()
        attn_ctx = ExitStack()
        attnw = ctx.enter_context(tc.tile_pool(name="attnw" + u, bufs=2))
        ptp = ctx.enter_context(tc.tile_pool(name="ptp" + u, bufs=3))
        ystp = ctx.enter_context(tc.tile_pool(name="ystp" + u, bufs=3))
        spp = attn_ctx.enter_context(tc.tile_pool(name="spp" + u, bufs=2, space="PSUM"))
        avp = attn_ctx.enter_context(tc.tile_pool(name="avp" + u, bufs=2, space="PSUM"))

        # ---- phase 3: attention, per (head, query-half) ----
        for h in range(HLOC):
            mt, half = divmod(h, 2)
            po = 64 * half
            for qh in range(2):
                glo = 1024 * qh
                kmax = 8 * (qh + 1)
                av = avp.tile([65, 2, 512], F32, tag="avp")
                for i in range(kmax):
                    q0 = 128 * i  # first unmasked query column for this k-tile
                    lo_g = max(glo, q0)
                    pt = ptp.tile([128, 1024], BF, tag="pt")
                    sp = spp.tile([128, 1024], F32, tag="spp")
                    has_diag = glo <= q0 < glo + 1024
                    for jj in range(2):
                        j = 2 * qh + jj
                        lo = max(512 * j, q0)
                        hi = 512 * (j + 1)
                        if lo >= hi:
                            continue
                        diag_bank = has_diag and (q0 - glo) // 512 == jj
                        nc.tensor.matmul(
                            sp[:, lo - glo : hi - glo],
                            lhsT=kT[po : po + 64, mt, 128 * i : 128 * (i + 1)],
                            rhs=qT[po : po + 64, mt, lo:hi],
                            start=True,
                            stop=not diag_bank,
                        )
                        if diag_bank:
                            # block-causal mask: scores -= 8192*disallowed
                            nc.tensor.matmul(
                                sp[:, q0 - glo : q0 - glo + 128],
                                lhsT=mu_sb,
                                rhs=mv_sb,
                                start=False,
                                stop=True,
                            )
                    # P^T = exp(rrms_k[k]/sqrt(HD) * scores - B0)
                    nc.scalar.activation(
                        pt[:, lo_g - glo : 1024],
                        sp[:, lo_g - glo : 1024],
                        A.Exp,
                        bias=b0_sb,
                        scale=rkz[:, h, i : i + 1],
                    )
                    # attn^T accumulation (+ denominator in row 64)
                    for jj in range(2):
                        j = 2 * qh + jj
                        jlo = max(512 * j, q0)
                        jhi = 512 * (j + 1)
                        if jlo >= jhi:
                            continue
                        nc.tensor.matmul(
                            av[:, jj, jlo - 512 * j : 512],
                            lhsT=vv[:, i, h, :],
                            rhs=pt[:, jlo - glo : jhi - glo],
                            start=(i == 0),
                            stop=(i == min(kmax, 4 * j + 4) - 1),
                        )
                # normalize: at[head rows] = av[0:64] * (1 / av[64])
                rden = attnw.tile([1, 2, 512], F32, tag="rden")
                nc.vector.reciprocal(rden, av[64:65])
                rdb = attnw.tile([64, 2, 512], F32, tag="rdb")
                nc.gpsimd.partition_broadcast(rdb, rden, channels=64)
                for jj in range(2):
                    nc.vector.tensor_mul(
                        at[
                            po : po + 64,
                            mt,
                            glo + 512 * jj : glo + 512 * (jj + 1),
                        ],
                        av[0:64, jj, :],
                        rdb[:, jj, :],
                    )

        # ---- phase 4: partial out-projection y^T = wo^T @ at ----
        attn_ctx.close()
        pp = ctx.enter_context(tc.tile_pool(name="pp2" + u, bufs=2, space="PSUM"))
        for m in range(8):
            ps = pp.tile([128, NST, 512], F32, tag="pp")
            for j in range(NST):
                for kt in range(2):
                    nc.tensor.matmul(
                        ps[:, j, :],
                        lhsT=wo_sb[:, kt, 128 * m : 128 * (m + 1)],
                        rhs=at[:, kt, 512 * j : 512 * (j + 1)],
                        start=(kt == 0),
                        stop=(kt == 1),
                    )
            yst = ystp.tile([128, NST, 512], F32, tag="yst")
            nc.vector.tensor_copy(yst[:, 0:2, :], ps[:, 0:2, :])
            nc.scalar.copy(yst[:, 2:4, :], ps[:, 2:4, :])
            nc.sync.dma_start(
                out=yt_d[4 * m : 4 * (m + 1)].rearrange("a p b -> p a b"),
                in_=yst,
            )


class _pin_act_table:
    """Context: force every activation we use (Exp, Ln, Copy) onto the one
    table set containing them all, so the program does a single
    ACT_TABLE_LOAD instead of thrashing natural_log <-> exp_and_others per
    RMS-norm pair. Restores the shared cached dict on exit."""

    def __init__(self, arch):
        from concourse.hw_specs import get_activation_tables

        self.tabs = get_activation_tables(arch)

    def __enter__(self):
        self.saved = {nm: set(s) for nm, s in self.tabs.items()}
        for nm, s in self.tabs.items():
            if nm != "natural_log_exp_and_others":
                s.clear()

    def __exit__(self, *a):
        for nm, s in self.tabs.items():
            s.clear()
            s.update(self.saved[nm])


def build_program(iters=1):
    nc = bacc.Bacc(
        "TRN2",
        target_bir_lowering=False,
        debug=False,
        enable_asserts=False,
        num_devices=NCORES,
    )
    with tile.TileContext(nc) as tc:
        io = _declare_io(nc)
        for it in range(iters):
            _emit(tc, io, u=f"_i{it}" if iters > 1 else "")
    with _pin_act_table(nc.m.arch):
        nc.compile()
    return nc


def make_core_inputs(x, qkv_w, out_w, qn_w, kn_w, rope_cos, rope_sin, attention_mask):
    """Host-side shard/layout prep. Returns list of 8 per-core input dicts."""
    x = np.asarray(x, np.float32)
    qkv_w = np.asarray(qkv_w, np.float32)
    out_w = np.asarray(out_w, np.float32)
    qn_w = np.asarray(qn_w, np.float32)
    kn_w = np.asarray(kn_w, np.float32)
    rope_cos = np.asarray(rope_cos, np.float32)
    rope_sin = np.asarray(rope_sin, np.float32)
    am = np.asarray(attention_mask)

    r = qkv_w.reshape(3, H, HD, D)
    csT = rope_cos.T.astype(np.float32)                # (64, S)
    snT = rope_sin.T.astype(np.float32)
    s2 = np.concatenate([-snT[0:32], snT[32:64]], axis=0)  # sign-folded sin
    perm = np.concatenate([np.arange(32, 64), np.arange(0, 32)])

    def fold(tab, w, permute):
        ww = w[perm] if permute else w
        t = tab * ww[:, None]
        return np.concatenate([t, t], axis=0).astype(BF16)  # (128, S)

    csq = fold(csT, qn_w, False)
    snq = fold(s2, qn_w, True)
    csk = fold(csT, kn_w, False)
    snk = fold(s2, kn_w, True)

    # rank-8 factorization of the (128,128) diagonal-block mask
    dis = ~(am[0:128, 0:128].T)                        # dis[k', q'] disallowed
    mu = np.zeros((8, 128), np.float32)
    mv = np.zeros((8, 128), np.float32)
    for t in range(8):
        mu[t] = np.arange(128) // 16 == t
        mv[t] = -MASK_C * dis[16 * t, :]
    ones2 = np.zeros((128, 2), np.float32)
    ones2[0:64, 0] = 1.0
    ones2[64:128, 1] = 1.0
    b0 = float(HD * SCALE * max(1e-30, np.abs(qn_w).max() * np.abs(kn_w).max()))
    b0_t = np.full((128, 1), -b0, np.float32)

    shared = dict(
        csq=csq,
        snq=snq,
        csk=csk,
        snk=snk,
        mu=mu.astype(BF16),
        mv=mv.astype(BF16),
        ones2=ones2.astype(BF16),
        b0=b0_t,
    )
    in_maps = []
    for c in range(NCORES):
        b, g = divmod(c, 4)
        hs = slice(HLOC * g, HLOC * (g + 1))
        m = dict(shared)
        m["xt"] = np.ascontiguousarray(x[b].T).astype(BF16)
        def _wlayout(w):
            # (D, M) -> (128, NDK*M): partition p holds [t, m] = w[t*128+p, m]
            mm = w.shape[1]
            return np.ascontiguousarray(
                w.reshape(-1, 128, mm).transpose(1, 0, 2).reshape(128, -1)
            ).astype(BF16)

        m["wq"] = _wlayout(r[0, hs].transpose(2, 0, 1).reshape(D, 256))
        m["wk"] = _wlayout(r[1, hs].transpose(2, 0, 1).reshape(D, 256))
        m["wv"] = _wlayout(r[2, hs].transpose(2, 0, 1).reshape(D, 256))
        m["wo"] = _wlayout(
            np.ascontiguousarray(out_w[:, 256 * g : 256 * (g + 1)].T)
        )
        in_maps.append(m)
    return in_maps


_PROGRAM = []


def get_program():
    if not _PROGRAM:
        _PROGRAM.append(build_program())
    return _PROGRAM[0]


def unshard(results):
    """results: list of 8 dicts with 'yt' (1024, 2048) fp32 partials."""
    ys = []
    for b in range(B):
        acc = np.zeros((32, 128, 512), np.float64)
        for g in range(4):
            acc += np.asarray(results[4 * b + g]["yt"], np.float32)
        yt = acc.reshape(8, 4, 128, 512).transpose(0, 2, 1, 3).reshape(D, S)
        ys.append(yt.T.astype(np.float32))
    return np.stack(ys)


def kernel(**inputs):
    in_maps = make_core_inputs(**inputs)
    nc = get_program()
    res = run_bass_kernel_spmd(nc, in_maps, core_ids=list(range(NCORES)))
    return unshard(res.results)


# revision 23
# speedup vs baseline: 19.6695x; 19.6695x over previous
"""Block-causal attention (B=2, S=2048, D=1024, H=16, HD=64, BLOCK=16) on 8 TRN2 cores.

Sharding: core c -> batch c//4, head-group c%4 (4 heads). Each core computes the
full attention for its 4 heads plus a partial out-projection y^T (1024, 2048);
the host sums the 4 partials per batch (row-parallel unshard) and transposes.

Device dataflow (per core) is fully "transposed":
  - qkv proj emits q^T/k^T in (head-dim, seq) layout, V in (seq, head-dim).
  - RMS-norm on q^T/k^T: squares on ACT, partition-sum via ones-matmul on PE,
    rsqrt as exp(-0.5*ln(.)) so Ln/Exp/Square share one ACT table set.
  - scores^T = (K^T-tile).T @ Q^T per head; block-causal mask is added inside
    the PE accumulation as a rank-8 (-720 * disallowed) matmul; exp needs no
    row-max because |scores| <= 8 after RMS norm (host passes the bound).
  - softmax denominator comes free: V carries an appended ones column (M=65).
  - attn^T = [V|1].T @ P^T accumulated over k-tiles; normalize by the
    reciprocal of row 64.
"""

import numpy as np
import ml_dtypes

import concourse.bass as bass
import concourse.tile as tile
from concourse import bacc
from concourse import mybir
from concourse.bass_utils import run_bass_kernel_spmd

BF16 = ml_dtypes.bfloat16
F32 = mybir.dt.float32
BF = mybir.dt.bfloat16

B, S, D, H, HD = 2, 2048, 1024, 16, 64
HLOC = 4          # heads per core
NCORES = 8
EPS = 1e-6
SCALE = HD ** -0.5
MASK_C = 8192.0   # masked-pair score offset; exp underflows to 0.0
NST = 4           # 512-wide seq tiles
NKT = 16          # 128-wide key tiles
NDK = 8           # 128-wide model-dim tiles


def _declare_io(nc):
    def din(name, shape, d=BF):
        return nc.dram_tensor(name, shape, d, kind="ExternalInput").ap()

    io = dict(
        xt_d=din("xt", [D, S]),
        wq_d=din("wq", [128, NDK * 256]),
        wk_d=din("wk", [128, NDK * 256]),
        wv_d=din("wv", [128, NDK * 256]),
        wo_d=din("wo", [128, 2 * D]),
        csq_d=din("csq", [128, S]),
        snq_d=din("snq", [128, S]),
        csk_d=din("csk", [128, S]),
        snk_d=din("snk", [128, S]),
        mu_d=din("mu", [8, 128]),
        mv_d=din("mv", [8, 128]),
        ones2_d=din("ones2", [128, 2]),
        b0_d=din("b0", [128, 1], F32),
        yt_d=nc.dram_tensor(
            "yt", [32, 128, 512], F32, kind="ExternalOutput"
        ).ap(),
    )
    return io


import os
_ABL = os.environ.get("BASS_ABL", "")


def _emit(tc, io, u=""):
    """Emit the per-core program. Pure SPMD: identical on all 8 cores.
    `u` suffixes pool names so the body can be emitted multiple times
    (benchmarking builds)."""
    from contextlib import ExitStack

    nc = tc.nc
    A = mybir.ActivationFunctionType
    OP = mybir.AluOpType
    xt_d = io["xt_d"]
    wq_d = io["wq_d"]
    wk_d = io["wk_d"]
    wv_d = io["wv_d"]
    wo_d = io["wo_d"]
    csq_d = io["csq_d"]
    snq_d = io["snq_d"]
    csk_d = io["csk_d"]
    snk_d = io["snk_d"]
    mu_d = io["mu_d"]
    mv_d = io["mv_d"]
    ones2_d = io["ones2_d"]
    b0_d = io["b0_d"]
    yt_d = io["yt_d"]

    ctx = ExitStack()
    proj_ctx = ExitStack()
    with ctx:
        consts = ctx.enter_context(tc.tile_pool(name="consts" + u, bufs=1))
        persist = ctx.enter_context(tc.tile_pool(name="persist" + u, bufs=1))
        dscratch = ctx.enter_context(tc.tile_pool(name="dscratch" + u, bufs=1, space="DRAM"))
        xtp = proj_ctx.enter_context(tc.tile_pool(name="xtp" + u, bufs=1))
        work2 = proj_ctx.enter_context(tc.tile_pool(name="work2" + u, bufs=2))
        sqp = proj_ctx.enter_context(tc.tile_pool(name="sqp" + u, bufs=3))
        pp = proj_ctx.enter_context(tc.tile_pool(name="pp" + u, bufs=2, space="PSUM"))
        vp = proj_ctx.enter_context(tc.tile_pool(name="vp" + u, bufs=2, space="PSUM"))
        msp = proj_ctx.enter_context(tc.tile_pool(name="msp" + u, bufs=1, space="PSUM"))

        # ---- input loads: x^T first (everything contracts over it), the
        # weights/tables on the gpsimd queue so both DMA channels fill ----
        xt_sb = xtp.tile([128, NDK, S], BF)
        for kt in range(NDK):
            nc.sync.dma_start(
                out=xt_sb[:, kt, :], in_=xt_d[128 * kt : 128 * (kt + 1), :]
            )
        wq_sb = consts.tile([128, NDK, 256], BF)
        wk_sb = consts.tile([128, NDK, 256], BF)
        wv_sb = consts.tile([128, NDK, 256], BF)
        wo_sb = consts.tile([128, 2, D], BF)
        nc.gpsimd.dma_start(out=wv_sb, in_=wv_d.rearrange("p (t m) -> p t m", t=NDK))
        nc.gpsimd.dma_start(out=wq_sb, in_=wq_d.rearrange("p (t m) -> p t m", t=NDK))
        nc.gpsimd.dma_start(out=wk_sb, in_=wk_d.rearrange("p (t m) -> p t m", t=NDK))
        csq_sb = consts.tile([128, S], BF)
        snq_sb = consts.tile([128, S], BF)
        csk_sb = consts.tile([128, S], BF)
        snk_sb = consts.tile([128, S], BF)
        nc.gpsimd.dma_start(out=csq_sb, in_=csq_d)
        nc.gpsimd.dma_start(out=snq_sb, in_=snq_d)
        nc.gpsimd.dma_start(out=csk_sb, in_=csk_d)
        nc.gpsimd.dma_start(out=snk_sb, in_=snk_d)
        nc.gpsimd.dma_start(out=wo_sb, in_=wo_d.rearrange("p (t m) -> p t m", t=2))
        mu_sb = consts.tile([8, 128], BF)
        mv_sb = consts.tile([8, 128], BF)
        nc.sync.dma_start(out=mu_sb, in_=mu_d)
        nc.sync.dma_start(out=mv_sb, in_=mv_d)
        ones2_sb = consts.tile([128, 2], BF)
        nc.sync.dma_start(out=ones2_sb, in_=ones2_d)
        b0_sb = consts.tile([128, 1], F32)
        nc.sync.dma_start(out=b0_sb, in_=b0_d)
        eps_sb = consts.tile([128, 1], F32)
        nc.vector.memset(eps_sb, EPS)

        # ---- persistent activations ----
        qT = persist.tile([128, 2, S], BF)      # (2 heads)*64 rows per m-tile
        kT = persist.tile([128, 2, S], BF)
        vv = persist.tile([128, NKT, HLOC, HD + 1], BF)   # [V | ones]
        at = persist.tile([128, 2, S], BF)      # normalized attn^T
        # pair p's two rows live at partition 32*p (engines need 32-aligned
        # start partitions)
        ln8 = persist.tile([98, NST, 512], F32)
        rr8 = persist.tile([98, NST, 512], BF)
        rkb = persist.tile([128, 64], BF)    # k-side rrms, (k mod 128, h*16+i)
        rkz = persist.tile([128, 4, 16], F32)  # SCALE * rrms_k per (head, ktile)
        rr_dram = dscratch.tile([8, 16, 128], BF)

        nc.vector.memset(vv[:, :, :, HD : HD + 1], 1.0)

        # ---- phase 1: V projection (s, d) ----
        for st in range(NKT):  # 128-row seq tiles
            ps = vp.tile([128, 256], F32, tag="vp")
            for kt in range(NDK):
                nc.tensor.matmul(
                    ps,
                    lhsT=xt_sb[:, kt, 128 * st : 128 * (st + 1)],
                    rhs=wv_sb[:, kt, :],
                    start=(kt == 0),
                    stop=(kt == NDK - 1),
                )
            nc.vector.tensor_copy(
                vv[:, st, :, 0:HD], ps.rearrange("p (h d) -> p h d", h=HLOC)
            )

        # ---- phase 2: Q/K projection + RMS norm + RoPE, (d, s) layout ----
        # RoPE commutes with the per-position rrms scale, and qn_w/kn_w are
        # folded into the cos/sin tables on the host, so the rotation DMA and
        # the rope muls run directly on the raw projection output; rrms is
        # applied last (Q side only -- the K side folds it into the exp
        # scale during attention).
        for qk_i, (wsb, cstab, sntab, dest) in enumerate(
            [(wq_sb, csq_sb, snq_sb, qT), (wk_sb, csk_sb, snk_sb, kT)]
        ):
            for mt in range(2):
                pair = qk_i * 2 + mt
                qraw = work2.tile([128, S], BF, tag="qraw")
                ms = msp.tile([2, NST, 512], F32, tag="msp")
                for st in range(NST):
                    ps = pp.tile([128, 512], F32, tag="pp")
                    for kt in range(NDK):
                        nc.tensor.matmul(
                            ps,
                            lhsT=wsb[:, kt, 128 * mt : 128 * (mt + 1)],
                            rhs=xt_sb[:, kt, 512 * st : 512 * (st + 1)],
                            start=(kt == 0),
                            stop=(kt == NDK - 1),
                        )
                    sl = slice(512 * st, 512 * (st + 1))
                    nc.vector.tensor_copy(qraw[:, sl], ps)
                    sq = sqp.tile([128, 512], BF, tag="sq")
                    nc.vector.tensor_mul(sq, qraw[:, sl], qraw[:, sl])
                    nc.tensor.matmul(
                        ms[:, st, :], lhsT=ones2_sb, rhs=sq, start=True, stop=True
                    )
                # rrms = exp(-0.5 * ln(ms/HD + eps)) for the 2 heads
                pb = 32 * pair
                nc.scalar.activation(
                    ln8[pb : pb + 2],
                    ms,
                    A.Ln,
                    bias=eps_sb[0:2],
                    scale=1.0 / HD,
                )
                nc.scalar.activation(
                    rr8[pb : pb + 2],
                    ln8[pb : pb + 2],
                    A.Exp,
                    scale=-0.5,
                )
                # rope on raw values (tables carry qn/kn and the sign fold)
                rot = work2.tile([128, S], BF, tag="rot")
                for lo, hi in ((0, 32), (32, 64), (64, 96), (96, 128)):
                    src_lo = lo + 32 if (lo // 32) % 2 == 0 else lo - 32
                    nc.sync.dma_start(
                        out=rot[lo:hi], in_=qraw[src_lo : src_lo + 32]
                    )
                t1 = work2.tile([128, S], BF, tag="t1")
                t2 = work2.tile([128, S], BF, tag="t2")
                nc.vector.tensor_mul(t1, qraw, cstab)
                nc.vector.tensor_mul(t2, rot, sntab)
                nc.sync.dma_start(
                    out=rr_dram[2 * pair : 2 * pair + 2].rearrange(
                        "r a b -> r (a b)"
                    ),
                    in_=rr8[pb : pb + 2].rearrange("p a b -> p (a b)"),
                )
                if qk_i == 1:
                    # K side: done after the add; rrms_k applied at exp time
                    nc.vector.tensor_add(dest[:, mt, :], t1, t2)
                    # k-side rrms rows -> partition-major via DMA transpose,
                    # folding in the 1/sqrt(HD) softmax scale
                    nc.sync.dma_start_transpose(
                        rkb[:, 32 * mt : 32 * (mt + 1)],
                        rr_dram[2 * pair : 2 * pair + 2].rearrange(
                            "r a b -> (r a) b"
                        ),
                    )
                    nc.vector.tensor_scalar_mul(
                        rkz[:, 2 * mt : 2 * mt + 2, :].rearrange(
                            "p h i -> p (h i)"
                        ),
                        rkb[:, 32 * mt : 32 * (mt + 1)],
                        SCALE,
                    )
                else:
                    tsum = work2.tile([128, S], BF, tag="tsum")
                    nc.vector.tensor_add(tsum, t1, t2)
                    # broadcast rrms across each head's 64 partitions
                    # (via DRAM: zero-step partition reads need a DRAM source)
                    rrb = work2.tile([128, NST, 512], BF, tag="rrb")
                    if _ABL == "nobcast":
                        nc.vector.memset(rrb, 1.0)
                    else:
                        nc.gpsimd.dma_start(
                            out=rrb[0:64],
                            in_=rr_dram[2 * pair : 2 * pair + 1]
                            .rearrange("r a b -> r (a b)")
                            .rearrange("r (a b) -> r a b", a=NST)
                            .partition_broadcast(64),
                        )
                        nc.gpsimd.dma_start(
                            out=rrb[64:128],
                            in_=rr_dram[2 * pair + 1 : 2 * pair + 2]
                            .rearrange("r a b -> r (a b)")
                            .rearrange("r (a b) -> r a b", a=NST)
                            .partition_broadcast(64),
                        )
                    for st in range(NST):
                        sl = slice(512 * st, 512 * (st + 1))
                        nc.vector.tensor_mul(
                            dest[:, mt, sl], tsum[:, sl], rrb[:, st, :]
                        )

        # proj scratch (incl. x^T) is dead now; free SBUF/PSUM for attention
        proj_ctx.close()
        attn_ctx = ExitStack()
        attnw = ctx.enter_context(tc.tile_pool(name="attnw" + u, bufs=2))
        ptp = ctx.enter_context(tc.tile_pool(name="ptp" + u, bufs=3))
        ystp = ctx.enter_context(tc.tile_pool(name="ystp" + u, bufs=3))
        spp = attn_ctx.enter_context(tc.tile_pool(name="spp" + u, bufs=2, space="PSUM"))
        avp = attn_ctx.enter_context(tc.tile_pool(name="avp" + u, bufs=2, space="PSUM"))

        # ---- phase 3: attention, per (head, query-half) ----
        for h in range(HLOC):
            mt, half = divmod(h, 2)
            po = 64 * half
            for qh in range(2):
                glo = 1024 * qh
                kmax = 8 * (qh + 1)
                av = avp.tile([65, 2, 512], F32, tag="avp")
                for i in range(kmax):
                    q0 = 128 * i  # first unmasked query column for this k-tile
                    lo_g = max(glo, q0)
                    pt = ptp.tile([128, 1024], BF, tag="pt")
                    sp = spp.tile([128, 1024], F32, tag="spp")
                    has_diag = glo <= q0 < glo + 1024
                    for jj in range(2):
                        j = 2 * qh + jj
                        lo = max(512 * j, q0)
                        hi = 512 * (j + 1)
                        if lo >= hi:
                            continue
                        diag_bank = has_diag and (q0 - glo) // 512 == jj
                        nc.tensor.matmul(
                            sp[:, lo - glo : hi - glo],
                            lhsT=kT[po : po + 64, mt, 128 * i : 128 * (i + 1)],
                            rhs=qT[po : po + 64, mt, lo:hi],
                            start=True,
                            stop=not diag_bank,
                        )
                        if diag_bank:
                            # block-causal mask: scores -= 8192*disallowed
                            nc.tensor.matmul(
                                sp[:, q0 - glo : q0 - glo + 128],
                                lhsT=mu_sb,
                                rhs=mv_sb,
                                start=False,
                                stop=True,
                            )
                    # P^T = exp(rrms_k[k]/sqrt(HD) * scores - B0)
                    nc.scalar.activation(
                        pt[:, lo_g - glo : 1024],
                        sp[:, lo_g - glo : 1024],
                        A.Exp,
                        bias=b0_sb,
                        scale=rkz[:, h, i : i + 1],
                    )
                    # attn^T accumulation (+ denominator in row 64)
                    for jj in range(2):
                        j = 2 * qh + jj
                        jlo = max(512 * j, q0)
                        jhi = 512 * (j + 1)
                        if jlo >= jhi:
                            continue
                        nc.tensor.matmul(
                            av[:, jj, jlo - 512 * j : 512],
                            lhsT=vv[:, i, h, :],
                            rhs=pt[:, jlo - glo : jhi - glo],
                            start=(i == 0),
                            stop=(i == min(kmax, 4 * j + 4) - 1),
                        )
                # normalize: at[head rows] = av[0:64] * (1 / av[64])
                rden = attnw.tile([1, 2, 512], F32, tag="rden")
                nc.vector.reciprocal(rden, av[64:65])
                rdb = attnw.tile([64, 2, 512], F32, tag="rdb")
                if _ABL == "nobcast":
                    nc.vector.memset(rdb, 1.0)
                else:
                    nc.gpsimd.partition_broadcast(rdb, rden, channels=64)
                for jj in range(2):
                    nc.vector.tensor_mul(
                        at[
                            po : po + 64,
                            mt,
                            glo + 512 * jj : glo + 512 * (jj + 1),
                        ],
                        av[0:64, jj, :],
                        rdb[:, jj, :],
                    )

        # ---- phase 4: partial out-projection y^T = wo^T @ at ----
        attn_ctx.close()
        pp = ctx.enter_context(tc.tile_pool(name="pp2" + u, bufs=2, space="PSUM"))
        for m in range(8):
            ps = pp.tile([128, NST, 512], F32, tag="pp")
            for j in range(NST):
                for kt in range(2):
                    nc.tensor.matmul(
                        ps[:, j, :],
                        lhsT=wo_sb[:, kt, 128 * m : 128 * (m + 1)],
                        rhs=at[:, kt, 512 * j : 512 * (j + 1)],
                        start=(kt == 0),
                        stop=(kt == 1),
                    )
            yst = ystp.tile([128, NST, 512], F32, tag="yst")
            nc.vector.tensor_copy(yst[:, 0:2, :], ps[:, 0:2, :])
            nc.scalar.copy(yst[:, 2:4, :], ps[:, 2:4, :])
            nc.sync.dma_start(
                out=yt_d[4 * m : 4 * (m + 1)].rearrange("a p b -> p a b"),
                in_=yst,
            )


class _pin_act_table:
    """Context: force every activation we use (Exp, Ln, Copy) onto the one
    table set containing them all, so the program does a single
    ACT_TABLE_LOAD instead of thrashing natural_log <-> exp_and_others per
    RMS-norm pair. Restores the shared cached dict on exit."""

    def __init__(self, arch):
        from concourse.hw_specs import get_activation_tables

        self.tabs = get_activation_tables(arch)

    def __enter__(self):
        self.saved = {nm: set(s) for nm, s in self.tabs.items()}
        for nm, s in self.tabs.items():
            if nm != "natural_log_exp_and_others":
                s.clear()

    def __exit__(self, *a):
        for nm, s in self.tabs.items():
            s.clear()
            s.update(self.saved[nm])


def build_program(iters=1):
    nc = bacc.Bacc(
        "TRN2",
        target_bir_lowering=False,
        debug=False,
        enable_asserts=False,
        num_devices=NCORES,
    )
    with tile.TileContext(nc) as tc:
        io = _declare_io(nc)
        for it in range(iters):
            _emit(tc, io, u=f"_i{it}" if iters > 1 else "")
    with _pin_act_table(nc.m.arch):
        nc.compile()
    return nc


def make_core_inputs(x, qkv_w, out_w, qn_w, kn_w, rope_cos, rope_sin, attention_mask):
    """Host-side shard/layout prep. Returns list of 8 per-core input dicts."""
    x = np.asarray(x, np.float32)
    qkv_w = np.asarray(qkv_w, np.float32)
    out_w = np.asarray(out_w, np.float32)
    qn_w = np.asarray(qn_w, np.float32)
    kn_w = np.asarray(kn_w, np.float32)
    rope_cos = np.asarray(rope_cos, np.float32)
    rope_sin = np.asarray(rope_sin, np.float32)
    am = np.asarray(attention_mask)

    r = qkv_w.reshape(3, H, HD, D)
    csT = rope_cos.T.astype(np.float32)                # (64, S)
    snT = rope_sin.T.astype(np.float32)
    s2 = np.concatenate([-snT[0:32], snT[32:64]], axis=0)  # sign-folded sin
    perm = np.concatenate([np.arange(32, 64), np.arange(0, 32)])

    def fold(tab, w, permute):
        ww = w[perm] if permute else w
        t = tab * ww[:, None]
        return np.concatenate([t, t], axis=0).astype(BF16)  # (128, S)

    csq = fold(csT, qn_w, False)
    snq = fold(s2, qn_w, True)
    csk = fold(csT, kn_w, False)
    snk = fold(s2, kn_w, True)

    # rank-8 factorization of the (128,128) diagonal-block mask
    dis = ~(am[0:128, 0:128].T)                        # dis[k', q'] disallowed
    mu = np.zeros((8, 128), np.float32)
    mv = np.zeros((8, 128), np.float32)
    for t in range(8):
        mu[t] = np.arange(128) // 16 == t
        mv[t] = -MASK_C * dis[16 * t, :]
    ones2 = np.zeros((128, 2), np.float32)
    ones2[0:64, 0] = 1.0
    ones2[64:128, 1] = 1.0
    b0 = float(HD * SCALE * max(1e-30, np.abs(qn_w).max() * np.abs(kn_w).max()))
    b0_t = np.full((128, 1), -b0, np.float32)

    shared = dict(
        csq=csq,
        snq=snq,
        csk=csk,
        snk=snk,
        mu=mu.astype(BF16),
        mv=mv.astype(BF16),
        ones2=ones2.astype(BF16),
        b0=b0_t,
    )
    in_maps = []
    for c in range(NCORES):
        b, g = divmod(c, 4)
        hs = slice(HLOC * g, HLOC * (g + 1))
        m = dict(shared)
        m["xt"] = np.ascontiguousarray(x[b].T).astype(BF16)
        def _wlayout(w):
            # (D, M) -> (128, NDK*M): partition p holds [t, m] = w[t*128+p, m]
            mm = w.shape[1]
            return np.ascontiguousarray(
                w.reshape(-1, 128, mm).transpose(1, 0, 2).reshape(128, -1)
            ).astype(BF16)

        m["wq"] = _wlayout(r[0, hs].transpose(2, 0, 1).reshape(D, 256))
        m["wk"] = _wlayout(r[1, hs].transpose(2, 0, 1).reshape(D, 256))
        m["wv"] = _wlayout(r[2, hs].transpose(2, 0, 1).reshape(D, 256))
        m["wo"] = _wlayout(
            np.ascontiguousarray(out_w[:, 256 * g : 256 * (g + 1)].T)
        )
        in_maps.append(m)
    return in_maps


_PROGRAM = []


def get_program():
    if not _PROGRAM:
        _PROGRAM.append(build_program())
    return _PROGRAM[0]


def unshard(results):
    """results: list of 8 dicts with 'yt' (1024, 2048) fp32 partials."""
    ys = []
    for b in range(B):
        acc = np.zeros((32, 128, 512), np.float64)
        for g in range(4):
            acc += np.asarray(results[4 * b + g]["yt"], np.float32)
        yt = acc.reshape(8, 4, 128, 512).transpose(0, 2, 1, 3).reshape(D, S)
        ys.append(yt.T.astype(np.float32))
    return np.stack(ys)


def kernel(**inputs):
    in_maps = make_core_inputs(**inputs)
    nc = get_program()
    res = run_bass_kernel_spmd(nc, in_maps, core_ids=list(range(NCORES)))
    return unshard(res.results)
